# revision 36
# baseline (speedup 1.0000x reference)
"""Bahdanau-style attention kernel for Trainium2 (8 NeuronCores, data-parallel).

Computes, for each batch b:
    h_proj = hidden @ w_h^T + attn_b                  # [H]
    e_proj = enc[b] @ w_e^T                           # [L, H]
    energy = tanh(h_proj + e_proj)                    # [L, H]
    scores = energy @ v_w                             # [L]
    weights = softmax(scores)                         # [L]
    context[b] = weights @ enc[b]                     # [H]

Sharding: data-parallel over batch B=32 across 8 cores (4 batches/core).
Params are replicated. The softmax max-subtraction is skipped (scores are
bounded by sum|v| <= 32, exp is safe in fp32); the 1/Z normalization is
folded into the final context scaling.

The dominant GEMM (e_proj) runs in fp8 e4m3 with perf_mode=DoubleRow
(2 fp8 weights per PE cell, 2 MACs/cycle). enc is pre-scaled by 16 and
w_e by 8192 on the host so both fit the e4m3 grid clear of the +-240 TRN
limit; the 2^-17 descale is folded into the tanh activation's scale.

Schedule: batch-INTERLEAVED slab groups. Iteration g streams the energy
GEMM for all 4 batches of l-tile g back-to-back on the PE. The tiny M=1
matmuls (scores partition-reduce "ones-MM" and the context accumulation)
for all 4 batches run CONCURRENTLY as column-tiled strips
(tile_position=(0,32b)) so they cost ~1 matmul of PE time instead of 4.
One exp activation per group covers all 4 batches (rows 0/32/64/96 of a
[128,512] PSUM tile), so the ACT queue is never head-of-line blocked.

Context accumulation shares one PSUM bank per h-half across the 4
batches (disjoint partition rows). Because a start=True matmul may clear
has_written state beyond its own elements, the banks are instead
initialized once by zeroing dummy matmuls during warmup and every
context matmul uses start=False (overwrite-where-unwritten semantics
make lc=0 a plain write and lc>=1 accumulate, on any hardware
interpretation of the clear granularity).

Built on bacc.Bacc so compile() runs the TRN2 wait-splitting passes.
"""

import numpy as np

H = 1024
B = 32
L = 2048
NCORES = 8
BPC = B // NCORES          # batches per core = 4
KC = H // 128              # contraction chunks of 128 = 8
KC2 = H // 256             # DoubleRow contraction chunks of 256 = 4
OC = H // 128              # output-feature chunks = 8
NLT = L // 512             # l-tiles of 512 = 4 (= slab groups)
NLCH = L // 128            # l-chunks of 128 = 16

ENC_SCALE = 16.0           # enc pre-scale before e4m3 quantization
W_SCALE = 8192.0           # w_e pre-scale before e4m3 quantization
DESCALE = 1.0 / (ENC_SCALE * W_SCALE)   # folded into tanh activation

_CACHED_NC = None


def _build_kernel():
    from contextlib import ExitStack

    import concourse.tile as tile
    from concourse import bacc
    from concourse import mybir
    from concourse.masks import make_identity

    f32 = mybir.dt.float32
    bf16 = mybir.dt.bfloat16
    fp8 = mybir.dt.float8e4
    AF = mybir.ActivationFunctionType
    DR = mybir.MatmulPerfMode.DoubleRow

    nc = bacc.Bacc("TRN2", target_bir_lowering=False, debug=False,
                   num_devices=NCORES)

    # all inputs host-laid-out so every DMA is contiguous per partition
    encT = nc.dram_tensor("encTr", [BPC, 128, NLT, KC, 512], fp8,
                          kind="ExternalInput").ap()
    encN = nc.dram_tensor("encNr", [BPC, 128, NLT, 4, H], bf16,
                          kind="ExternalInput").ap()
    w_eT = nc.dram_tensor("wer", [128, KC, H], fp8, kind="ExternalInput").ap()
    smallr = nc.dram_tensor("smallr", [128, OC + OC * BPC], f32,
                            kind="ExternalInput").ap()
    ctx_out = nc.dram_tensor("ctx", [BPC, H], f32, kind="ExternalOutput").ap()
    # DRAM bounce for the exp(scores) transpose: full [128,512] tile per
    # group (big DMA packets), read back through the DMA XBAR transpose
    escr = nc.dram_tensor("escr", [NLT, 128, 512], bf16).ap()

    with tile.TileContext(nc) as tc, ExitStack() as ctx:
        consts = ctx.enter_context(tc.tile_pool(name="consts", bufs=1))
        encT_pool = ctx.enter_context(tc.tile_pool(name="encT", bufs=8))
        encN_pool = ctx.enter_context(tc.tile_pool(name="encN", bufs=8))
        en_pool = ctx.enter_context(tc.tile_pool(name="energy", bufs=4))
        acc_pool = ctx.enter_context(tc.tile_pool(name="accp", bufs=8))
        small = ctx.enter_context(tc.tile_pool(name="small", bufs=2))
        expwT_pool = ctx.enter_context(tc.tile_pool(name="expwT", bufs=2))

        # ---- constants / prologue DMAs ----
        # The first slab runs k2-outer over o-quads, so consumption order is
        # round r = {w_e[2r], w_e[2r+1] (o<4 halves), encT-b0 chunk r}.  Emit
        # prologue DMAs in exactly that priority order, round-robined across
        # the 3 DMA queues so arrivals track consumption (~95 GB/s each).
        small_sb = consts.tile([128, OC + OC * BPC], f32)
        v_sb = small_sb[:, 0:OC]
        # h_proj + attn_b, host-folded: [128, OC, BPC]
        hproj_sb = small_sb[:, OC:].rearrange("p (o b) -> p o b", b=BPC)

        encTs = {}
        t00 = encT_pool.tile([128, KC, 512], fp8, tag="encTs", name="encTs00")
        encTs[(0, 0)] = t00
        we_sb = consts.tile([128, KC, H], fp8)           # w_e^T  [h-part, k, o]
        tb = {}
        for b in range(1, BPC):
            tb[b] = encT_pool.tile([128, KC, 512], fp8, tag="encTs",
                                   name=f"encTs{b}0")
            encTs[(b, 0)] = tb[b]

        # explicit queue assignment (S=sync, A=scalar, G=gpsimd), hand-
        # balanced so each queue's serial backlog tracks the k2-outer
        # consumption deadlines (~60-66 GB/s effective per queue)
        S, A, G = nc.sync, nc.scalar, nc.gpsimd
        pro_items = [(S, small_sb, smallr)]
        rot = [(A, G, S), (G, S, A), (S, A, G), (A, G, S)]
        for k2 in range(KC2):
            q0, q1, q2 = rot[k2]
            pro_items.append((q0, we_sb[:, 2 * k2, 0:512],
                              w_eT[:, 2 * k2, 0:512]))
            pro_items.append((q1, we_sb[:, 2 * k2 + 1, 0:512],
                              w_eT[:, 2 * k2 + 1, 0:512]))
            pro_items.append((q2, t00[:, 2 * k2:2 * k2 + 2, :],
                              encT[0, :, 0, 2 * k2:2 * k2 + 2, :]))
        # w_e o>=4 halves, then b1's slab split across two queues
        weBq = [G, S, A, G, S, A, G, S]
        for k in range(KC):
            pro_items.append((weBq[k], we_sb[:, k, 512:H], w_eT[:, k, 512:H]))
        pro_items.append((A, tb[1][:, 0:KC2, :], encT[1, :, 0, 0:KC2, :]))
        pro_items.append((G, tb[1][:, KC2:KC, :], encT[1, :, 0, KC2:KC, :]))
        pro_items.append((S, tb[2], encT[2, :, 0]))
        pro_items.append((A, tb[3], encT[3, :, 0]))
        for q, dst, src in pro_items:
            q.dma_start(out=dst, in_=src)

        ones_f32 = consts.tile([128, 32], f32)
        nc.vector.memset(ones_f32, 1.0)
        # [128,32] so each strip matmul fills its whole 32-row strip
        # (CoreSim requires fully-initialized PSUM reads; cost is N-driven)
        ones_bf = consts.tile([128, 32], bf16)
        nc.vector.tensor_copy(ones_bf, ones_f32)
        zeros_bf = consts.tile([128, 32], bf16)
        nc.vector.memset(zeros_bf, 0.0)
        zeros512_bf = consts.tile([128, 512], bf16)
        nc.vector.memset(zeros512_bf, 0.0)
        zacc4 = consts.tile([128, NLT], f32)             # per-group Z partials
        # identity for the last group's PE-transpose path
        ident = consts.tile([128, 128], f32)
        make_identity(nc, ident)
        ident_bf = consts.tile([128, 128], bf16)
        nc.vector.tensor_copy(ident_bf, ident)
        # v replicated to 32-wide strips: the last slab's v-reduce runs as PE
        # matmuls (M=32) instead of the serial DVE chain, cutting tail latency
        v32 = consts.tile([128, 32 * OC], bf16)
        for o in range(OC):
            nc.vector.tensor_scalar_mul(v32[:, 32 * o:32 * o + 32], ones_bf,
                                        v_sb[:, o:o + 1])
        # no PE warmup block: the cold start overlaps the initial DMA anyway,
        # and HAM reaches 8/8 ~3.4us into the first energy stream

        # PSUM: 5 banks ring for the energy GEMM (tanh drain lags ~1 group),
        # 1 bank for the 4-batch scores tile, 2 banks for the 4-batch context
        # accumulators (one per h-half, batches at partition rows 0/32/64/96).
        pp_e = ctx.enter_context(tc.tile_pool(name="pp_e", bufs=5, space="PSUM"))
        pp_s = ctx.enter_context(tc.tile_pool(name="pp_s", bufs=1, space="PSUM"))
        pp_c = ctx.enter_context(tc.tile_pool(name="pp_c", bufs=2, space="PSUM"))

        pcs = [pp_c.tile([128, 512], f32, tag="pc", name=f"pc{h}")
               for h in range(2)]
        # initialize the shared context banks: zero every batch row once, so
        # all later context matmuls can use start=False (see module docstring)
        for half in range(2):
            for b in range(BPC):
                nc.tensor.matmul(
                    pcs[half][32 * b:32 * b + 32, :], zeros_bf, zeros512_bf,
                    start=True, stop=False, tile_position=(0, 32 * b),
                    skip_group_check=True)
        # warm-keepers: the 8 dummies above run cold (~3.4us) and trip the
        # HAM to 8/8; these no-dep matmuls keep the PE busy while the
        # first slab's DMAs land, so the energy stream starts at 2.4 GHz
        pwarm = pp_e.tile([128, 512], f32, tag="pe", name="pwarm")
        for _ in range(6):
            nc.tensor.matmul(pwarm[0:32, :], zeros_bf, zeros512_bf,
                             start=True, stop=True, skip_group_check=True)

        state = {}

        def get_psc(g):
            # scores psum tile, allocated on first use (g=3's is first
            # touched by b3's in-line PE v-reduce, before emit_ones)
            if (g, "psc") not in state:
                state[(g, "psc")] = pp_s.tile([128, 512], f32, tag="psc",
                                              name=f"psc{g}")
            return state[(g, "psc")]

        def emit_vacc(b, en, o, acc):
            # accumulate v-weighted energy on DVE (partition-wise)
            if o == 0:
                nc.vector.tensor_scalar_mul(acc, en, v_sb[:, 0:1])
            else:
                nc.vector.scalar_tensor_tensor(
                    out=acc, in0=en, scalar=v_sb[:, o:o + 1], in1=acc,
                    op0=mybir.AluOpType.mult, op1=mybir.AluOpType.add)

        def emit_energy(b, g):
            ts = encTs.pop((b, g))
            # encN for slab (b,g): consumed by the context matmuls one
            # iteration later; 2 slabs each on the gpsimd/scalar queues
            encNs = encN_pool.tile([128, 4, H], bf16, tag="encNs",
                                   name=f"encNs{b}_{g}")
            eng = nc.gpsimd if b < 2 else nc.scalar
            eng.dma_start(out=encNs, in_=encN[b, :, g])
            state[(b, g, "encN")] = encNs
            acc = acc_pool.tile([128, 512], bf16, tag="acc", name=f"acc{b}_{g}")
            if (b, g) == (0, 0):
                # first slab: k2-outer over o-quads so compute starts as soon
                # as the first 128KB chunk + first w_e pair land
                for os_ in (0, 4):
                    pes = [pp_e.tile([128, 512], f32, tag="pe",
                                     name=f"pe0_{os_ + oi}")
                           for oi in range(4)]
                    for k2 in range(KC2):
                        for oi in range(4):
                            o = os_ + oi
                            nc.tensor.matmul(
                                pes[oi],
                                we_sb[:, 2 * k2:2 * k2 + 2,
                                      o * 128:(o + 1) * 128],
                                ts[:, 2 * k2:2 * k2 + 2, :],
                                start=(k2 == 0), stop=(k2 == KC2 - 1),
                                perf_mode=DR,
                            )
                    for oi in range(4):
                        o = os_ + oi
                        en = en_pool.tile([128, 512], bf16, tag="en")
                        nc.scalar.activation(en, pes[oi], AF.Tanh,
                                             scale=DESCALE,
                                             bias=hproj_sb[:, o, b:b + 1])
                        emit_vacc(b, en, o, acc)
            else:
                last = (b, g) == (BPC - 1, NLT - 1)
                for o in range(OC):
                    pe = pp_e.tile([128, 512], f32, tag="pe")
                    for k2 in range(KC2):
                        nc.tensor.matmul(
                            pe,
                            we_sb[:, 2 * k2:2 * k2 + 2, o * 128:(o + 1) * 128],
                            ts[:, 2 * k2:2 * k2 + 2, :],
                            start=(k2 == 0), stop=(k2 == KC2 - 1),
                            perf_mode=DR,
                        )
                    en = en_pool.tile([128, 512], bf16, tag="en")
                    nc.scalar.activation(en, pe, AF.Tanh, scale=DESCALE,
                                         bias=hproj_sb[:, o, b:b + 1])
                    if last:
                        # last slab: v-reduce as accumulating PE strip
                        # matmuls; each depends only on its own tanh, so the
                        # tail skips the serial DVE chain entirely
                        nc.tensor.matmul(
                            get_psc(g)[96:128, :],
                            v32[:, 32 * o:32 * o + 32], en,
                            start=(o == 0), stop=(o == OC - 1),
                            tile_position=(0, 96), skip_group_check=True)
                    else:
                        emit_vacc(b, en, o, acc)
            if not ((b, g) == (BPC - 1, NLT - 1)):
                state[(b, g, "acc")] = acc

        def emit_prefetch(b, g):
            # encT slab (b,g) prefetch on the sync queue during iteration g-1
            t = encT_pool.tile([128, KC, 512], fp8, tag="encTs",
                               name=f"encTs{b}{g}")
            nc.sync.dma_start(out=t, in_=encT[b, :, g])
            encTs[(b, g)] = t

        def emit_ones(g, bs):
            # scores partition-reduce, column-tiled strips:
            # psc[32b:32b+32, :] = ones^T @ acc_b  (concurrent)
            psc = get_psc(g)
            for b in bs:
                nc.tensor.matmul(
                    psc[32 * b:32 * b + 32, :], ones_bf,
                    state.pop((b, g, "acc")),
                    start=True, stop=True, tile_position=(0, 32 * b),
                    skip_group_check=True)

        def emit_exp(g):
            # one exp covers all 4 batches (rows 0/32/64/96); Z rides the
            # activation accumulator into zacc4[:, g]
            psc = state.pop((g, "psc"))
            expw4 = small.tile([128, 512], bf16, tag="expw4", name=f"expw4{g}")
            nc.scalar.activation(expw4, psc, AF.Exp,
                                 accum_out=zacc4[:, g:g + 1])
            expwT4 = expwT_pool.tile([128, 4, 128], bf16, tag="expwT4",
                                     name=f"expwT4{g}")
            if g == NLT - 1:
                # tail: transpose on the PE (~1.5us chain) instead of the
                # DMA round trip (~6us) — the PE is idle here anyway
                tp = pp_s.tile([128, 4, 256], bf16, tag="psc", name="tp3")
                for j in range(4):
                    nc.tensor.transpose(tp[:, j, 0:128],
                                        expw4[:, 128 * j:128 * (j + 1)],
                                        ident_bf)
                nc.vector.tensor_copy(expwT4, tp[:, :, 0:128])
            else:
                # steady state: full-tile DRAM bounce (big packets) +
                # per-l-chunk XBAR transpose loads; context group j only
                # waits on its own 128-column block
                nc.sync.dma_start(out=escr[g], in_=expw4)
                for j in range(4):
                    nc.sync.dma_start_transpose(
                        out=expwT4[:, j, :],
                        in_=escr[g, :, 128 * j:128 * (j + 1)],
                    )
            state[(g, "expwT4")] = expwT4

        def emit_ctx(g, js):
            expwT4 = state[(g, "expwT4")]
            for j in js:
                lc = 4 * g + j
                for half in range(2):
                    for b in range(BPC):
                        nc.tensor.matmul(
                            pcs[half][32 * b:32 * b + 1, :],
                            expwT4[:, j, 32 * b:32 * b + 1],
                            state[(b, g, "encN")][:, j,
                                                  half * 512:(half + 1) * 512],
                            start=False, stop=(lc == NLCH - 1),
                            tile_position=(0, 32 * b),
                            skip_group_check=True)
            if js[-1] == 3:
                state.pop((g, "expwT4"))
                for b in range(BPC):
                    state.pop((b, g, "encN"))

        # ---- main loop: 4 slab groups, 4 batches each ----
        for g in range(NLT):
            for b in range(BPC):
                emit_energy(b, g)
                # the exp->transpose chain is emitted BEFORE the encT
                # prefetch so it sits at the head of the sync queue (a 512KB
                # prefetch ahead of it would add ~5us to the chain latency)
                if g > 0 and b == 0:
                    emit_ones(g - 1, range(BPC))
                    emit_exp(g - 1)
                if g < NLT - 1:
                    emit_prefetch(b, g + 1)
                if g > 0:
                    if b == 1:
                        emit_ctx(g - 1, [0, 1])
                    elif b == 2:
                        emit_ctx(g - 1, [2, 3])

        # ---- epilogue: last group's scores/context + finalize ----
        g = NLT - 1
        emit_ones(g, range(BPC - 1))
        emit_exp(g)
        emit_ctx(g, [0, 1, 2, 3])

        zs = small.tile([128, 1], f32, tag="zs")
        nc.vector.reduce_sum(zs, zacc4, axis=mybir.AxisListType.X)
        rz = small.tile([128, 1], f32, tag="rz")
        nc.vector.reciprocal(rz, zs)
        for half in range(2):
            out4 = small.tile([128, 512], f32, tag=f"out4_{half}")
            nc.vector.tensor_scalar_mul(out4, pcs[half], rz)
            eng = nc.sync if half == 0 else nc.gpsimd
            eng.dma_start(out=ctx_out[:, half * 512:(half + 1) * 512],
                          in_=out4[0:128:32, :])

    nc.compile()
    return nc


def _get_nc():
    global _CACHED_NC
    if _CACHED_NC is None:
        _CACHED_NC = _build_kernel()
    return _CACHED_NC


def _make_in_maps(hidden, encoder_outputs, attn_w, attn_b, v_w):
    import ml_dtypes

    e4m3 = ml_dtypes.float8_e4m3

    hidden = np.asarray(hidden, dtype=np.float32)
    encoder_outputs = np.asarray(encoder_outputs, dtype=np.float32)
    attn_w = np.asarray(attn_w, dtype=np.float32)
    attn_b = np.asarray(attn_b, dtype=np.float32)
    v_w = np.asarray(v_w, dtype=np.float32)

    wer32 = np.ascontiguousarray(
        attn_w[:, H:].T.reshape(KC, 128, H).transpose(1, 0, 2))
    wer = np.clip(wer32 * W_SCALE, -240.0, 240.0).astype(e4m3)
    # fold the tiny h_proj = hidden @ w_h^T + b into a per-core bias input
    hproj_pb = hidden @ attn_w[:, :H].T + attn_b     # [B, H]

    enc8_full = np.clip(encoder_outputs * ENC_SCALE, -240.0, 240.0).astype(e4m3)

    in_maps = []
    for c in range(NCORES):
        sl = slice(c * BPC, (c + 1) * BPC)
        enc = encoder_outputs[sl]                       # [BPC, L, H]
        # encTr[b, p, lt, k, l] = q(enc[b, lt*512 + l, k*128 + p] * 16)
        encTr = np.ascontiguousarray(
            enc8_full[sl].reshape(BPC, NLT, 512, KC, 128)
            .transpose(0, 4, 1, 3, 2))
        # encNr[b, p, lt, j, h] = enc[b, lt*512 + j*128 + p, h]  (bf16)
        encNr = np.ascontiguousarray(
            enc.reshape(BPC, NLT, 4, 128, H).transpose(0, 3, 1, 2, 4)
            .astype(ml_dtypes.bfloat16))
        # smallr: [v chunks | h_proj+b chunks]  (hp[p, o, b] layout)
        hp = hproj_pb[sl].T.reshape(OC, 128, BPC).transpose(1, 0, 2)
        smallr = np.concatenate([
            v_w.reshape(OC, 128).T,
            hp.reshape(128, OC * BPC),
        ], axis=1)
        in_maps.append({
            "encTr": encTr,
            "encNr": encNr,
            "wer": wer,
            "smallr": np.ascontiguousarray(smallr),
        })
    return in_maps


def kernel(hidden, encoder_outputs, attn_w, attn_b, v_w):
    from concourse.bass_utils import run_bass_kernel_spmd

    in_maps = _make_in_maps(hidden, encoder_outputs, attn_w, attn_b, v_w)
    nc = _get_nc()
    res = run_bass_kernel_spmd(nc, in_maps, list(range(NCORES)))
    out = np.concatenate([res.results[c]["ctx"] for c in range(NCORES)], axis=0)
    return out.astype(np.float32)
